# revision 20
# baseline (speedup 1.0000x reference)
"""Trainium2 Bass kernel for nn_DotPred (gnn_message_passing).

score[t, e] = sum_d (x[src] - x[dst]) / sqrt(D)
            = sP[src] + sN[dst],  sP = rowsum(x)/sqrt(D), sN = -sP

Strategy (8 NeuronCores, SPMD, one program):
- Phase 1: node rows sharded 8x: each core row-sums its 12544-row slice of
  node_embeds on DVE, scales by 1/sqrt(D); an AllGather replicates the full
  100352-entry table; it is laid out in SBUF as [128, 784] bf16 with node n
  at (n % 128, col(n)), plus a negated copy (one [128, 1600] table).
- Phase 2: the 3M endpoint lookups (src and dst sides of 1.5M edges,
  sharded 8x) are one-hot matmul gathers. Per side, edges are sorted by
  128-node table column; a static per-column tile schedule (derived from
  global counts) is shared by all 8 cores. Per 128-edge tile:
    PE poly matmul (k=6, exact bf16 integer decomposition):
        Q[p, e] = 2 p*pe - p^2 - pe^2 = -(p - pe)^2
    relu(1 + Q) -> exact one-hot OHP [128, 128] (split DVE/ACT/Pool)
    PE select matmul: lhsT = OHP (stationary), rhs = table column ->
        psum val column [128 edges, 1]; src tiles read +s cols, dst tiles
        read -s cols.
- All value arithmetic (row sums, scaling, sign, lookups) happens on
  device; the host only sorts/pads edge indices and, when unsharding,
  reassembles score[e] from the two per-edge device partials.
"""
import math
from contextlib import ExitStack

import numpy as np

import concourse.bass as bass
import concourse.mybir as mybir
from concourse.bass_utils import run_bass_kernel_spmd

P = 128
D = 128
NCORES = 8
N_NODES = 100000
RPC = 12544              # node rows per core (= 128 * 98)
JCOLS = RPC // P         # 98
NPAD = RPC * NCORES      # 100352
NBLK = NCORES * JCOLS    # 784 table columns (128 nodes each)
TBL_COLS = 1600          # [0,784): +s ; [800,1584): -s
TPB = 16                 # tiles per batch (2048 edge slots)
CHT = 4                  # tiles per poly chunk (512 edges)
NCH = TPB // CHT
INV_SQ = 1.0 / math.sqrt(float(D))
EMB_SPLIT = 2
N_OHP = 20               # OHP batch buffers (prologue overlap + lag)
LAG = 17                 # select lag behind polys (batches)
VB = 512                 # psum val-bank width (tiles per bank fill)
VBB = VB // TPB          # batches per bank fill (32)
P3G = 4                  # batches per p3 DMA group
N_P3B = 3                # p3 group buffers

F32 = mybir.dt.float32
BF16 = mybir.dt.bfloat16
ALU = mybir.AluOpType
ACTF = mybir.ActivationFunctionType

# relu engine split by "super" (2 poly chunks = 1024 cols): early batches
# avoid Pool (busy with the collective) and go easy on DVE (phase 1);
# steady state balances DVE/ACT/Pool by modeled throughput.
EARLY_B = 16
SUP_B = 2                    # supers per batch
PAT_EARLY = ["a", "v"]
PAT_LATE = ["a", "v", "a", "v", "a", "a", "v", "a", "v", "a", "a", "v"]


def _relu_engine(r):
    if r // SUP_B < EARLY_B:
        return PAT_EARLY[r % SUP_B]
    return PAT_LATE[(r - EARLY_B * SUP_B) % len(PAT_LATE)]


def _relu_cum(r):
    eng = _relu_engine(r)
    n = sum(1 for rr in range(r + 1) if _relu_engine(rr) == eng)
    return eng, n


def _build_nc(sched):
    """sched: table-column index per tile (len = n_tiles, multiple of TPB)."""
    n_tiles = len(sched)
    assert n_tiles % (TPB * P3G) == 0
    nb = n_tiles // TPB
    ngrp = nb // P3G
    nfill = (n_tiles + VB - 1) // VB
    JC = JCOLS // EMB_SPLIT

    nc = bass.Bass(num_devices=NCORES)
    embeds = nc.declare_dram_parameter("embeds", [RPC, D], F32, isOutput=False)
    lhsT6_in = nc.declare_dram_parameter("lhsT6", [6, P], BF16, isOutput=False)
    p3_in = nc.declare_dram_parameter("p3", [ngrp, 6, P3G * TPB * P], BF16, isOutput=False)
    y = nc.declare_dram_parameter("y", [P, n_tiles], F32, isOutput=True)
    s_part = nc.dram_tensor("s_part", [RPC, 1], F32, kind="Internal")
    s_full = nc.dram_tensor("s_full", [NPAD, 1], F32, kind="Internal")

    es = ExitStack()
    with es:
        emb_sb = es.enter_context(nc.sbuf_tensor([P, RPC], F32))
        s_sb = es.enter_context(nc.sbuf_tensor([P, JCOLS], F32))
        tabf = es.enter_context(nc.sbuf_tensor([P, NBLK], F32))
        tab = es.enter_context(nc.sbuf_tensor([P, TBL_COLS], BF16))
        lhsT6 = es.enter_context(nc.sbuf_tensor([6, P], BF16))
        p3bufs = [
            es.enter_context(
                nc.sbuf_tensor(f"p3buf{b}", [6, P3G * TPB * P], BF16)
            )
            for b in range(N_P3B)
        ]
        ohp = es.enter_context(nc.sbuf_tensor([P, N_OHP * TPB * P], BF16))
        val_sb = es.enter_context(nc.sbuf_tensor([P, n_tiles], F32))
        q0 = es.enter_context(nc.psum_tensor([P, 2 * CHT * P], F32))
        q1 = es.enter_context(nc.psum_tensor([P, 2 * CHT * P], F32))
        q2 = es.enter_context(nc.psum_tensor([P, 2 * CHT * P], F32))
        vps0 = es.enter_context(nc.psum_tensor([P, VB], F32))
        vps1 = es.enter_context(nc.psum_tensor([P, VB], F32))
        s_embs = [
            es.enter_context(nc.semaphore(f"s_emb{k}")) for k in range(2)
        ]
        s_red = es.enter_context(nc.semaphore())
        s_sp = es.enter_context(nc.semaphore())
        s_ag = es.enter_context(nc.semaphore())
        s_sfl = es.enter_context(nc.semaphore())
        s_tab = es.enter_context(nc.semaphore())
        s_pre = es.enter_context(nc.semaphore())
        s_p3s = [
            es.enter_context(nc.semaphore(f"s_p3{b}")) for b in range(N_P3B)
        ]
        s_poly = es.enter_context(nc.semaphore())
        s_rv = es.enter_context(nc.semaphore())
        s_ra = es.enter_context(nc.semaphore())
        s_rp = es.enter_context(nc.semaphore())
        s_sel = es.enter_context(nc.semaphore())
        s_vcp = es.enter_context(nc.semaphore())
        s_y = es.enter_context(nc.semaphore())
        block = es.enter_context(nc.Block())

        qb = [q0, q1, q2]
        vps = [vps0, vps1]
        RS = {"v": s_rv, "a": s_ra, "p": s_rp}

        def relu_counts_through(i):
            return {
                e: sum(
                    1
                    for rr in range((i + 1) * SUP_B)
                    if _relu_engine(rr) == e
                )
                for e in "vap"
            }

        def ohp_sup_slice(r):
            i = r // SUP_B
            ob = i % N_OHP
            h = r % SUP_B
            return ohp[:, (ob * TPB + h * 2 * CHT) * P:
                       (ob * TPB + (h + 1) * 2 * CHT) * P]

        def relu_waits(eng_obj, r):
            i = r // SUP_B
            eng_obj.wait_ge(s_poly, 2 * (r + 1))
            if i >= N_OHP:
                eng_obj.wait_ge(s_sel, i - N_OHP + 1)

        def emit_selects(tensor, i):
            if i == 0:
                tensor.wait_ge(s_tab, 2)
            for e, n in relu_counts_through(i).items():
                if n:
                    tensor.wait_ge(RS[e], n)
            fill = (i * TPB) // VB
            if fill >= 2:
                tensor.wait_ge(s_vcp, fill - 1)
            ob = i % N_OHP
            for j in range(TPB):
                t = i * TPB + j
                mm = tensor.matmul(
                    out=vps[(t // VB) % 2][:, t % VB:t % VB + 1],
                    lhsT=ohp[:, (ob * TPB + j) * P:(ob * TPB + j + 1) * P],
                    rhs=tab[:, sched[t]:sched[t] + 1],
                    start=True,
                    stop=True,
                )
                if j == TPB - 1:
                    mm.then_inc(s_sel, 1)

        @block.sync
        def _(sync):
            sync.dma_start(out=lhsT6[:], in_=lhsT6_in[:]).then_inc(s_pre, 16)
            for k in range(EMB_SPLIT):
                sync.dma_start(
                    out=emb_sb[:, k * JC * D:(k + 1) * JC * D],
                    in_=embeds[k * JC * P:(k + 1) * JC * P, :].rearrange(
                        "(j p) d -> p j d", p=P
                    ),
                ).then_inc(s_embs[k % 2], 16)
            for g in range(ngrp):
                if g >= N_P3B:
                    sync.wait_ge(s_poly, NCH * P3G * (g - N_P3B + 1))
                sync.dma_start(
                    out=p3bufs[g % N_P3B][:], in_=p3_in[g]
                ).then_inc(s_p3s[g % N_P3B], 16)
            sync.wait_ge(s_vcp, nfill)
            sync.dma_start(out=y[:], in_=val_sb[:]).then_inc(s_y, 16)

        @block.gpsimd
        def _(gpsimd):
            gpsimd.wait_ge(s_red, EMB_SPLIT + 1)
            gpsimd.dma_start(
                out=s_part[:, 0].rearrange("(p j) -> p j", p=P), in_=s_sb[:]
            ).then_inc(s_sp, 16)
            gpsimd.wait_ge(s_sp, 16)
            gpsimd.collective_compute(
                "AllGather",
                ALU.bypass,
                replica_groups=[list(range(NCORES))],
                ins=[s_part[:, 0]],
                outs=[s_full[:, 0]],
            ).then_inc(s_ag, 1)
            gpsimd.wait_ge(s_ag, 1)
            gpsimd.dma_start(
                out=tabf[:],
                in_=s_full[:, 0].rearrange("(c p j) -> p c j", p=P, j=JCOLS),
            ).then_inc(s_sfl, 16)
            gpsimd.wait_ge(s_y, 16)

        @block.scalar
        def _(scalar):
            vcp = 0
            for i in range(nb):
                for h in range(SUP_B):
                    r = i * SUP_B + h
                    if _relu_engine(r) != "a":
                        continue
                    relu_waits(scalar, r)
                    scalar.activation(
                        out=ohp_sup_slice(r),
                        in_=qb[r % 3][:],
                        func=ACTF.Relu,
                        bias=1.0,
                        scale=1.0,
                    ).then_inc(s_ra, 1)
                if vcp < nfill - 1 and i == VBB * (vcp + 1) + LAG + 3:
                    lo, hi = vcp * VB, (vcp + 1) * VB
                    scalar.wait_ge(s_sel, hi // TPB)
                    scalar.copy(
                        out=val_sb[:, lo:hi], in_=vps[vcp % 2][:]
                    ).then_inc(s_vcp, 1)
                    vcp += 1
            while vcp < nfill:
                lo, hi = vcp * VB, min(n_tiles, (vcp + 1) * VB)
                scalar.wait_ge(s_sel, (hi + TPB - 1) // TPB)
                scalar.copy(
                    out=val_sb[:, lo:hi], in_=vps[vcp % 2][:, 0:hi - lo]
                ).then_inc(s_vcp, 1)
                vcp += 1

        @block.vector
        def _(vector):
            for k in range(EMB_SPLIT):
                vector.wait_ge(s_embs[k % 2], 16 * (k // 2 + 1))
                vector.tensor_reduce(
                    out=s_sb[:, k * JC:(k + 1) * JC],
                    in_=emb_sb[:, k * JC * D:(k + 1) * JC * D].rearrange(
                        "p (j d) -> p j d", d=D
                    ),
                    op=ALU.add,
                    axis=mybir.AxisListType.X,
                ).then_inc(s_red, 1)
            vector.wait_ge(s_red, EMB_SPLIT)
            vector.tensor_scalar(
                out=s_sb[:], in0=s_sb[:], scalar1=INV_SQ, scalar2=None,
                op0=ALU.mult,
            ).then_inc(s_red, 1)


            def vrelus(i):
                for h in range(SUP_B):
                    r = i * SUP_B + h
                    if _relu_engine(r) != "v":
                        continue
                    relu_waits(vector, r)
                    vector.tensor_scalar(
                        out=ohp_sup_slice(r),
                        in0=qb[r % 3][:],
                        scalar1=1.0,
                        scalar2=0.0,
                        op0=ALU.add,
                        op1=ALU.max,
                    ).then_inc(s_rv, 1)

            for i in range(min(EARLY_B, nb)):
                vrelus(i)
            vector.wait_ge(s_sfl, 16)
            vector.tensor_copy(out=tab[:, 0:NBLK], in_=tabf[:]).then_inc(
                s_tab, 1
            )
            vector.tensor_scalar(
                out=tab[:, 800:800 + NBLK], in0=tabf[:], scalar1=-1.0,
                scalar2=None, op0=ALU.mult,
            ).then_inc(s_tab, 1)
            for i in range(EARLY_B, nb):
                vrelus(i)
            vector.wait_ge(s_y, 16)

        @block.tensor
        def _(tensor):
            tensor.wait_ge(s_pre, 16)
            for i in range(nb):
                if i % P3G == 0:
                    g = i // P3G
                    tensor.wait_ge(s_p3s[g % N_P3B], 16 * (g // N_P3B + 1))
                for c in range(NCH):
                    q = i * NCH + c
                    r = q // 2
                    if c % 2 == 0 and r >= 3:
                        eng, n = _relu_cum(r - 3)
                        tensor.wait_ge(RS[eng], n)   # psum Q super free
                    tensor.matmul(
                        out=qb[r % 3][:, (q % 2) * CHT * P:
                                      (q % 2 + 1) * CHT * P],
                        lhsT=lhsT6[:],
                        rhs=p3bufs[(i // P3G) % N_P3B][
                            :, ((i % P3G) * TPB + c * CHT) * P:
                               ((i % P3G) * TPB + (c + 1) * CHT) * P],
                        start=True,
                        stop=True,
                    ).then_inc(s_poly, 1)
                if i >= LAG:
                    emit_selects(tensor, i - LAG)
            for i in range(max(0, nb - LAG), nb):
                emit_selects(tensor, i)
            tensor.wait_ge(s_y, 16)

    return nc


def _tbl_col(n):
    """Table column of node n (partition is n % 128)."""
    c, r = np.divmod(n, RPC)
    return c * JCOLS + r // P


def _prep_side(flat_idx, dst_side):
    """Global sort by table column; static per-core tile schedule."""
    n = flat_idx.astype(np.int64)
    col = _tbl_col(n)
    pe = (n % P).astype(np.int64)
    cnt = np.bincount(col, minlength=NBLK)
    tiles_per = (cnt + NCORES * P - 1) // (NCORES * P)
    n_tiles = int(tiles_per.sum())
    tile_base = np.zeros(NBLK, np.int64)
    np.cumsum(tiles_per[:-1], out=tile_base[1:])

    order = np.argsort(col, kind="stable")
    cstart = np.zeros(NBLK, np.int64)
    np.cumsum(cnt[:-1], out=cstart[1:])
    j_in_blk = np.arange(len(n), dtype=np.int64) - cstart[col[order]]
    core = j_in_blk % NCORES
    pos = j_in_blk // NCORES
    slot = tile_base[col[order]] * P + pos
    core_of = np.empty(len(n), np.int64)
    slot_of = np.empty(len(n), np.int64)
    core_of[order] = core
    slot_of[order] = slot

    sched = []
    off = 800 if dst_side else 0
    for b in range(NBLK):
        sched.extend([off + b] * int(tiles_per[b]))

    pe_slots = np.zeros((NCORES, n_tiles * P), np.int64)
    pe_slots[core_of, slot_of] = pe
    return sched, n_tiles, pe_slots, core_of, slot_of


def _p3_rows(pe_all, n_tiles):
    """Per-core [ngrp, 6, P3G*TPB*P] bf16 poly rhs rows incl. constant ones."""
    import ml_dtypes

    pe2 = pe_all * pe_all
    ones = np.ones_like(pe_all, dtype=np.float32)
    rows = np.stack(
        [
            pe_all.astype(np.float32),
            (pe2 >> 7).astype(np.float32),
            (pe2 & 127).astype(np.float32),
            ones,
            ones,
            ones,
        ],
        axis=2,
    )  # [core, slots, 6]
    ngrp = n_tiles // (TPB * P3G)
    r = rows.reshape(NCORES, ngrp, P3G * TPB * P, 6).transpose(0, 1, 3, 2)
    return np.ascontiguousarray(r).astype(ml_dtypes.bfloat16)


def _prep(src_flat, dst_flat):
    sched_s, nts, pes_s, core_s, slot_s = _prep_side(src_flat, False)
    sched_d, ntd, pes_d, core_d, slot_d = _prep_side(dst_flat, True)
    sched = sched_s + sched_d
    pad = (-len(sched)) % (TPB * P3G)
    sched += [0] * pad
    n_tiles = len(sched)
    pe_all = np.zeros((NCORES, n_tiles * P), np.int64)
    pe_all[:, :nts * P] = pes_s
    pe_all[:, nts * P:(nts + ntd) * P] = pes_d
    p3 = _p3_rows(pe_all, n_tiles)
    return sched, nts, p3, (core_s, slot_s), (core_d, slot_d)


def _lhsT6():
    import ml_dtypes

    p = np.arange(P, dtype=np.float32)
    a = np.floor(p / 16.0)
    b = p - 16.0 * a
    rows = np.stack([
        2.0 * p,
        np.full(P, -128.0, np.float32),
        np.full(P, -1.0, np.float32),
        -256.0 * a * a,
        -32.0 * a * b,
        -b * b,
    ])
    return rows.astype(ml_dtypes.bfloat16)


def kernel(node_embeds, src_idx, dst_idx):
    node_embeds = np.asarray(node_embeds, dtype=np.float32)
    src_idx = np.asarray(src_idx)
    dst_idx = np.asarray(dst_idx)
    T, E = src_idx.shape

    emb_pad = np.zeros((NPAD, D), np.float32)
    emb_pad[:node_embeds.shape[0]] = node_embeds

    src_flat = src_idx.reshape(-1).astype(np.int64)
    dst_flat = dst_idx.reshape(-1).astype(np.int64)
    sched, nts, p3, (core_s, slot_s), (core_d, slot_d) = _prep(
        src_flat, dst_flat
    )

    nc = _build_nc(sched)
    lh = _lhsT6()
    in_maps = []
    for i in range(NCORES):
        in_maps.append(
            {
                "embeds": emb_pad[i * RPC:(i + 1) * RPC],
                "lhsT6": lh,
                "p3": p3[i],
            }
        )
    res = run_bass_kernel_spmd(nc, in_maps, list(range(NCORES)))

    ys = [np.asarray(res.results[i]["y"]) for i in range(NCORES)]
    vs = np.empty(T * E, np.float32)
    vd = np.empty(T * E, np.float32)
    for i in range(NCORES):
        yf = ys[i].T.reshape(-1)        # slot-major: slot = t*128 + p
        m = core_s == i
        vs[m] = yf[slot_s[m]]
        m = core_d == i
        vd[m] = yf[nts * P + slot_d[m]]
    out_flat = vs + vd
    return out_flat.reshape(T, E)


# revision 24
# speedup vs baseline: 1.0379x; 1.0379x over previous
"""Trainium2 Bass kernel for nn_DotPred (gnn_message_passing).

score[t, e] = sum_d (x[src] - x[dst]) / sqrt(D)
            = sP[src] + sN[dst],  sP = rowsum(x)/sqrt(D), sN = -sP

Strategy (8 NeuronCores, SPMD, one program):
- Phase 1: node rows sharded 8x: each core row-sums its 12544-row slice of
  node_embeds on DVE, scales by 1/sqrt(D); an AllGather replicates the full
  100352-entry table; it is laid out in SBUF as [128, 784] bf16 with node n
  at (n % 128, col(n)), plus a negated copy (one [128, 1600] table).
- Phase 2: the 3M endpoint lookups (src and dst sides of 1.5M edges,
  sharded 8x) are one-hot matmul gathers. Per side, edges are sorted by
  128-node table column; a static per-column tile schedule (derived from
  global counts) is shared by all 8 cores. Per 128-edge tile:
    PE poly matmul (k=6, exact bf16 integer decomposition):
        Q[p, e] = 2 p*pe - p^2 - pe^2 = -(p - pe)^2
    relu(1 + Q) -> exact one-hot OHP [128, 128] (split DVE/ACT/Pool)
    PE select matmul: lhsT = OHP (stationary), rhs = table column ->
        psum val column [128 edges, 1]; src tiles read +s cols, dst tiles
        read -s cols.
- All value arithmetic (row sums, scaling, sign, lookups) happens on
  device; the host only sorts/pads edge indices and, when unsharding,
  reassembles score[e] from the two per-edge device partials.
"""
import math
from contextlib import ExitStack

import numpy as np

import concourse.bass as bass
import concourse.mybir as mybir
from concourse.bass_utils import run_bass_kernel_spmd

P = 128
D = 128
NCORES = 8
N_NODES = 100000
RPC = 12544              # node rows per core (= 128 * 98)
JCOLS = RPC // P         # 98
NPAD = RPC * NCORES      # 100352
NBLK = NCORES * JCOLS    # 784 table columns (128 nodes each)
TBL_COLS = 1600          # [0,784): +s ; [800,1584): -s
TPB = 16                 # tiles per batch (2048 edge slots)
CHT = 4                  # tiles per poly chunk (512 edges)
NCH = TPB // CHT
INV_SQ = 1.0 / math.sqrt(float(D))
EMB_SPLIT = 7
N_OHP = 20               # OHP batch buffers (prologue overlap + lag)
LAG = 17                 # select lag behind polys (batches)
VB = 512                 # psum val-bank width (tiles per bank fill)
VBB = VB // TPB          # batches per bank fill (32)
P3G = 4                  # batches per p3 DMA group
N_P3B = 3                # p3 group buffers

F32 = mybir.dt.float32
BF16 = mybir.dt.bfloat16
ALU = mybir.AluOpType
ACTF = mybir.ActivationFunctionType

# relu engine split by "super" (2 poly chunks = 1024 cols): early batches
# avoid Pool (busy with the collective) and go easy on DVE (phase 1);
# steady state balances DVE/ACT/Pool by modeled throughput.
EARLY_A = 10                 # batches: ACT-only (DVE in phase 1)
EARLY_B = 24                 # batches: alternating a/v warmup
SUP_B = 2                    # supers per batch
PAT_EARLY = ["a", "v"]
PAT_LATE = ["a", "v", "a", "v", "a", "v", "a", "v", "a", "v", "a", "v", "a"]


def _relu_engine(r):
    i = r // SUP_B
    if i < EARLY_A:
        return "a"
    if i < EARLY_B:
        return PAT_EARLY[r % SUP_B]
    return PAT_LATE[(r - EARLY_B * SUP_B) % len(PAT_LATE)]


def _relu_cum(r):
    eng = _relu_engine(r)
    n = sum(1 for rr in range(r + 1) if _relu_engine(rr) == eng)
    return eng, n


def _build_nc(sched):
    """sched: table-column index per tile (len = n_tiles, multiple of TPB)."""
    n_tiles = len(sched)
    assert n_tiles % (TPB * P3G) == 0
    nb = n_tiles // TPB
    ngrp = nb // P3G
    nfill = (n_tiles + VB - 1) // VB
    JC = JCOLS // EMB_SPLIT

    nc = bass.Bass(num_devices=NCORES)
    embeds = nc.declare_dram_parameter("embeds", [RPC, D], F32, isOutput=False)
    lhsT6_in = nc.declare_dram_parameter("lhsT6", [6, P], BF16, isOutput=False)
    p3_in = nc.declare_dram_parameter("p3", [ngrp, 6, P3G * TPB * P], BF16, isOutput=False)
    y = nc.declare_dram_parameter("y", [P, n_tiles], F32, isOutput=True)
    s_part = nc.dram_tensor("s_part", [RPC, 1], BF16, kind="Internal")
    s_full = nc.dram_tensor("s_full", [NPAD, 1], BF16, kind="Internal")

    es = ExitStack()
    with es:
        emb_sb = es.enter_context(nc.sbuf_tensor([P, RPC], F32))
        s_sb = es.enter_context(nc.sbuf_tensor([P, JCOLS], F32))
        s16_sb = es.enter_context(nc.sbuf_tensor([P, JCOLS], BF16))
        tabf = es.enter_context(nc.sbuf_tensor([P, NBLK], BF16))
        tab = es.enter_context(nc.sbuf_tensor([P, TBL_COLS], BF16))
        lhsT6 = es.enter_context(nc.sbuf_tensor([6, P], BF16))
        p3bufs = [
            es.enter_context(
                nc.sbuf_tensor(f"p3buf{b}", [6, P3G * TPB * P], BF16)
            )
            for b in range(N_P3B)
        ]
        ohp = es.enter_context(nc.sbuf_tensor([P, N_OHP * TPB * P], BF16))
        val_sb = es.enter_context(nc.sbuf_tensor([P, n_tiles], F32))
        q0 = es.enter_context(nc.psum_tensor([P, 2 * CHT * P], F32))
        q1 = es.enter_context(nc.psum_tensor([P, 2 * CHT * P], F32))
        q2 = es.enter_context(nc.psum_tensor([P, 2 * CHT * P], F32))
        vps0 = es.enter_context(nc.psum_tensor([P, VB], F32))
        vps1 = es.enter_context(nc.psum_tensor([P, VB], F32))
        s_embs = [
            es.enter_context(nc.semaphore(f"s_emb{k}")) for k in range(2)
        ]
        s_red = es.enter_context(nc.semaphore())
        s_sp = es.enter_context(nc.semaphore())
        s_ag = es.enter_context(nc.semaphore())
        s_sfl = es.enter_context(nc.semaphore())
        s_tab = es.enter_context(nc.semaphore())
        s_pre = es.enter_context(nc.semaphore())
        s_p3s = [
            es.enter_context(nc.semaphore(f"s_p3{b}")) for b in range(N_P3B)
        ]
        s_poly = es.enter_context(nc.semaphore())
        s_rv = es.enter_context(nc.semaphore())
        s_ra = es.enter_context(nc.semaphore())
        s_rp = es.enter_context(nc.semaphore())
        s_sel = es.enter_context(nc.semaphore())
        s_vcp = es.enter_context(nc.semaphore())
        s_y = es.enter_context(nc.semaphore())
        block = es.enter_context(nc.Block())

        qb = [q0, q1, q2]
        vps = [vps0, vps1]
        RS = {"v": s_rv, "a": s_ra, "p": s_rp}

        def relu_counts_through(i):
            return {
                e: sum(
                    1
                    for rr in range((i + 1) * SUP_B)
                    if _relu_engine(rr) == e
                )
                for e in "vap"
            }

        def ohp_sup_slice(r):
            i = r // SUP_B
            ob = i % N_OHP
            h = r % SUP_B
            return ohp[:, (ob * TPB + h * 2 * CHT) * P:
                       (ob * TPB + (h + 1) * 2 * CHT) * P]

        def relu_waits(eng_obj, r):
            i = r // SUP_B
            eng_obj.wait_ge(s_poly, 2 * (r + 1))
            if i >= N_OHP:
                eng_obj.wait_ge(s_sel, i - N_OHP + 1)

        def emit_selects(tensor, i):
            if i == 0:
                tensor.wait_ge(s_tab, 2)
            for e, n in relu_counts_through(i).items():
                if n:
                    tensor.wait_ge(RS[e], n)
            fill = (i * TPB) // VB
            if fill >= 2:
                tensor.wait_ge(s_vcp, fill - 1)
            ob = i % N_OHP
            for j in range(TPB):
                t = i * TPB + j
                mm = tensor.matmul(
                    out=vps[(t // VB) % 2][:, t % VB:t % VB + 1],
                    lhsT=ohp[:, (ob * TPB + j) * P:(ob * TPB + j + 1) * P],
                    rhs=tab[:, sched[t]:sched[t] + 1],
                    start=True,
                    stop=True,
                )
                if j == TPB - 1:
                    mm.then_inc(s_sel, 1)

        @block.sync
        def _(sync):
            sync.dma_start(out=lhsT6[:], in_=lhsT6_in[:]).then_inc(s_pre, 16)
            for g in range(ngrp):
                if g >= N_P3B:
                    sync.wait_ge(s_poly, NCH * P3G * (g - N_P3B + 1))
                sync.dma_start(
                    out=p3bufs[g % N_P3B][:], in_=p3_in[g]
                ).then_inc(s_p3s[g % N_P3B], 16)
                if g == N_P3B - 1:
                    for k in range(EMB_SPLIT):
                        sync.dma_start(
                            out=emb_sb[:, k * JC * D:(k + 1) * JC * D],
                            in_=embeds[
                                k * JC * P:(k + 1) * JC * P, :
                            ].rearrange("(j p) d -> p j d", p=P),
                        ).then_inc(s_embs[k % 2], 16)
            sync.wait_ge(s_vcp, nfill)
            sync.dma_start(out=y[:], in_=val_sb[:]).then_inc(s_y, 16)

        @block.gpsimd
        def _(gpsimd):
            gpsimd.wait_ge(s_red, EMB_SPLIT + 1)
            gpsimd.dma_start(
                out=s_part[:, 0].rearrange("(p j) -> p j", p=P), in_=s16_sb[:]
            ).then_inc(s_sp, 16)
            gpsimd.wait_ge(s_sp, 16)
            gpsimd.collective_compute(
                "AllGather",
                ALU.bypass,
                replica_groups=[list(range(NCORES))],
                ins=[s_part[:, 0]],
                outs=[s_full[:, 0]],
            ).then_inc(s_ag, 1)
            gpsimd.wait_ge(s_ag, 1)
            gpsimd.dma_start(
                out=tabf[:],
                in_=s_full[:, 0].rearrange("(c p j) -> p c j", p=P, j=JCOLS),
            ).then_inc(s_sfl, 16)
            gpsimd.wait_ge(s_y, 16)

        @block.scalar
        def _(scalar):
            vcp = 0
            for i in range(nb):
                for h in range(SUP_B):
                    r = i * SUP_B + h
                    if _relu_engine(r) != "a":
                        continue
                    relu_waits(scalar, r)
                    scalar.activation(
                        out=ohp_sup_slice(r),
                        in_=qb[r % 3][:],
                        func=ACTF.Relu,
                        bias=1.0,
                        scale=1.0,
                    ).then_inc(s_ra, 1)
                if vcp < nfill - 1 and i == VBB * (vcp + 1) + LAG + 3:
                    lo, hi = vcp * VB, (vcp + 1) * VB
                    scalar.wait_ge(s_sel, hi // TPB)
                    scalar.copy(
                        out=val_sb[:, lo:hi], in_=vps[vcp % 2][:]
                    ).then_inc(s_vcp, 1)
                    vcp += 1
            while vcp < nfill:
                lo, hi = vcp * VB, min(n_tiles, (vcp + 1) * VB)
                scalar.wait_ge(s_sel, (hi + TPB - 1) // TPB)
                scalar.copy(
                    out=val_sb[:, lo:hi], in_=vps[vcp % 2][:, 0:hi - lo]
                ).then_inc(s_vcp, 1)
                vcp += 1

        @block.vector
        def _(vector):
            for k in range(EMB_SPLIT):
                vector.wait_ge(s_embs[k % 2], 16 * (k // 2 + 1))
                vector.tensor_reduce(
                    out=s_sb[:, k * JC:(k + 1) * JC],
                    in_=emb_sb[:, k * JC * D:(k + 1) * JC * D].rearrange(
                        "p (j d) -> p j d", d=D
                    ),
                    op=ALU.add,
                    axis=mybir.AxisListType.X,
                ).then_inc(s_red, 1)
            vector.wait_ge(s_red, EMB_SPLIT)
            vector.tensor_scalar(
                out=s16_sb[:], in0=s_sb[:], scalar1=INV_SQ, scalar2=None,
                op0=ALU.mult,
            ).then_inc(s_red, 1)


            def vrelus(i):
                for h in range(SUP_B):
                    r = i * SUP_B + h
                    if _relu_engine(r) != "v":
                        continue
                    relu_waits(vector, r)
                    vector.tensor_scalar(
                        out=ohp_sup_slice(r),
                        in0=qb[r % 3][:],
                        scalar1=1.0,
                        scalar2=0.0,
                        op0=ALU.add,
                        op1=ALU.max,
                    ).then_inc(s_rv, 1)

            for i in range(min(EARLY_B, nb)):
                vrelus(i)
            vector.wait_ge(s_sfl, 16)
            vector.tensor_copy(out=tab[:, 0:NBLK], in_=tabf[:]).then_inc(
                s_tab, 1
            )
            vector.tensor_scalar(
                out=tab[:, 800:800 + NBLK], in0=tabf[:], scalar1=-1.0,
                scalar2=None, op0=ALU.mult,
            ).then_inc(s_tab, 1)
            for i in range(EARLY_B, nb):
                vrelus(i)
            vector.wait_ge(s_y, 16)

        @block.tensor
        def _(tensor):
            tensor.wait_ge(s_pre, 16)
            for i in range(nb):
                if i % P3G == 0:
                    g = i // P3G
                    tensor.wait_ge(s_p3s[g % N_P3B], 16 * (g // N_P3B + 1))
                for c in range(NCH):
                    q = i * NCH + c
                    r = q // 2
                    if c % 2 == 0 and r >= 3:
                        eng, n = _relu_cum(r - 3)
                        tensor.wait_ge(RS[eng], n)   # psum Q super free
                    tensor.matmul(
                        out=qb[r % 3][:, (q % 2) * CHT * P:
                                      (q % 2 + 1) * CHT * P],
                        lhsT=lhsT6[:],
                        rhs=p3bufs[(i // P3G) % N_P3B][
                            :, ((i % P3G) * TPB + c * CHT) * P:
                               ((i % P3G) * TPB + (c + 1) * CHT) * P],
                        start=True,
                        stop=True,
                    ).then_inc(s_poly, 1)
                if i >= LAG:
                    emit_selects(tensor, i - LAG)
            for i in range(max(0, nb - LAG), nb):
                emit_selects(tensor, i)
            tensor.wait_ge(s_y, 16)

    return nc


def _tbl_col(n):
    """Table column of node n (partition is n % 128)."""
    c, r = np.divmod(n, RPC)
    return c * JCOLS + r // P


def _prep_side(flat_idx, dst_side):
    """Global sort by table column; static per-core tile schedule."""
    n = flat_idx.astype(np.int64)
    col = _tbl_col(n)
    pe = (n % P).astype(np.int64)
    cnt = np.bincount(col, minlength=NBLK)
    tiles_per = (cnt + NCORES * P - 1) // (NCORES * P)
    n_tiles = int(tiles_per.sum())
    tile_base = np.zeros(NBLK, np.int64)
    np.cumsum(tiles_per[:-1], out=tile_base[1:])

    order = np.argsort(col, kind="stable")
    cstart = np.zeros(NBLK, np.int64)
    np.cumsum(cnt[:-1], out=cstart[1:])
    j_in_blk = np.arange(len(n), dtype=np.int64) - cstart[col[order]]
    core = j_in_blk % NCORES
    pos = j_in_blk // NCORES
    slot = tile_base[col[order]] * P + pos
    core_of = np.empty(len(n), np.int64)
    slot_of = np.empty(len(n), np.int64)
    core_of[order] = core
    slot_of[order] = slot

    sched = []
    off = 800 if dst_side else 0
    for b in range(NBLK):
        sched.extend([off + b] * int(tiles_per[b]))

    pe_slots = np.zeros((NCORES, n_tiles * P), np.int64)
    pe_slots[core_of, slot_of] = pe
    return sched, n_tiles, pe_slots, core_of, slot_of


def _p3_rows(pe_all, n_tiles):
    """Per-core [ngrp, 6, P3G*TPB*P] bf16 poly rhs rows incl. constant ones."""
    import ml_dtypes

    pe2 = pe_all * pe_all
    ones = np.ones_like(pe_all, dtype=np.float32)
    rows = np.stack(
        [
            pe_all.astype(np.float32),
            (pe2 >> 7).astype(np.float32),
            (pe2 & 127).astype(np.float32),
            ones,
            ones,
            ones,
        ],
        axis=2,
    )  # [core, slots, 6]
    ngrp = n_tiles // (TPB * P3G)
    r = rows.reshape(NCORES, ngrp, P3G * TPB * P, 6).transpose(0, 1, 3, 2)
    return np.ascontiguousarray(r).astype(ml_dtypes.bfloat16)


def _prep(src_flat, dst_flat):
    sched_s, nts, pes_s, core_s, slot_s = _prep_side(src_flat, False)
    sched_d, ntd, pes_d, core_d, slot_d = _prep_side(dst_flat, True)
    sched = sched_s + sched_d
    pad = (-len(sched)) % (TPB * P3G)
    sched += [0] * pad
    n_tiles = len(sched)
    pe_all = np.zeros((NCORES, n_tiles * P), np.int64)
    pe_all[:, :nts * P] = pes_s
    pe_all[:, nts * P:(nts + ntd) * P] = pes_d
    p3 = _p3_rows(pe_all, n_tiles)
    return sched, nts, p3, (core_s, slot_s), (core_d, slot_d)


def _lhsT6():
    import ml_dtypes

    p = np.arange(P, dtype=np.float32)
    a = np.floor(p / 16.0)
    b = p - 16.0 * a
    rows = np.stack([
        2.0 * p,
        np.full(P, -128.0, np.float32),
        np.full(P, -1.0, np.float32),
        -256.0 * a * a,
        -32.0 * a * b,
        -b * b,
    ])
    return rows.astype(ml_dtypes.bfloat16)


def kernel(node_embeds, src_idx, dst_idx):
    node_embeds = np.asarray(node_embeds, dtype=np.float32)
    src_idx = np.asarray(src_idx)
    dst_idx = np.asarray(dst_idx)
    T, E = src_idx.shape

    emb_pad = np.zeros((NPAD, D), np.float32)
    emb_pad[:node_embeds.shape[0]] = node_embeds

    src_flat = src_idx.reshape(-1).astype(np.int64)
    dst_flat = dst_idx.reshape(-1).astype(np.int64)
    sched, nts, p3, (core_s, slot_s), (core_d, slot_d) = _prep(
        src_flat, dst_flat
    )

    nc = _build_nc(sched)
    lh = _lhsT6()
    in_maps = []
    for i in range(NCORES):
        in_maps.append(
            {
                "embeds": emb_pad[i * RPC:(i + 1) * RPC],
                "lhsT6": lh,
                "p3": p3[i],
            }
        )
    res = run_bass_kernel_spmd(nc, in_maps, list(range(NCORES)))

    ys = [np.asarray(res.results[i]["y"]) for i in range(NCORES)]
    vs = np.empty(T * E, np.float32)
    vd = np.empty(T * E, np.float32)
    for i in range(NCORES):
        yf = ys[i].T.reshape(-1)        # slot-major: slot = t*128 + p
        m = core_s == i
        vs[m] = yf[slot_s[m]]
        m = core_d == i
        vd[m] = yf[nts * P + slot_d[m]]
    out_flat = vs + vd
    return out_flat.reshape(T, E)
